# revision 1
# baseline (speedup 1.0000x reference)
"""Trainium2 Bass kernel: MultiHeadAttention with QK-RMSNorm + partial rotary,
causal softmax. B=4, T=2048, D=1024, H=16, HD=64, fp32.

Sharding: 8 cores = 4 batches x 2 head-groups (8 heads each). Each core:
  - QKV projections for its batch, restricted to its 512 head-dims
  - causal attention for its 8 heads
  - partial output projection (its 512 contraction dims, all 1024 outputs)
Host sums the two head-group partials per batch (the all-reduce) and
transposes back.

Layout: fully transposed pipeline, zero on-chip transposes:
  xt [D, T] -> Qt/Kt [hd, t] (proj with wT as lhsT), V [t, hd]
  scores St[j, i] = Kt^T-row . Qt-col  (transposed scores, causal over j<=i)
  softmax without max-subtraction (RMS-normed q,k bound |s| <= 8)
  AV: lhsT = [V | ones] (j, 65), rhs = exp(St) -> Ot [65, i] with
      row 64 = softmax denominator (free)
  out-proj: lhsT = woT chunk, rhs = normalized Ot -> yt [dout, t]
Matmuls run as float32r (full-rate fp32 on the PE at N>=256).
"""

import numpy as np
from contextlib import ExitStack

import concourse.bass as bass
import concourse.tile as tile
import concourse.mybir as mybir
from concourse import bacc

F32 = mybir.dt.float32
MM_DT = mybir.dt.float32r # float32r = full-rate; float32 = exact, 1/4-rate
AF = mybir.ActivationFunctionType
MULT = mybir.AluOpType.mult
ADD = mybir.AluOpType.add

D = 1024   # model dim
DH = 512   # head-group width per core (8 heads x 64)
NH = 8     # heads per core
HD = 64    # head dim
NKC = D // 128   # k-chunks over model dim
EPS = 1e-6


def _r(ap):
    return ap.bitcast(MM_DT)


def _cast_dve(nc, ap):
    """In-place round f32 -> f32r (DVE) so walrus accepts it as mm input."""
    nc.vector.tensor_copy(ap.bitcast(MM_DT), ap)


def _cast_act(nc, ap):
    nc.scalar.copy(ap.bitcast(MM_DT), ap)


def build_kernel(nc: bass.Bass, T: int = 2048, dbg: bool = False):
    """Trace the per-core program. T parameterized for fast sim smoke tests."""
    NTT = T // 512     # 512-wide t/i blocks
    NTS = T // 128     # 128-wide t/j chunks

    if dbg:
        dqt = nc.dram_tensor("dqt", [4, 128, T], F32, kind="ExternalOutput").ap()
        dkt = nc.dram_tensor("dkt", [4, 128, T], F32, kind="ExternalOutput").ap()
        dqr = nc.dram_tensor("dqr", [4, 128, T], F32, kind="ExternalOutput").ap()
        dv = nc.dram_tensor("dv", [NTS, 128, NH * 65], F32,
                            kind="ExternalOutput").ap()
        dp = nc.dram_tensor("dp", [2, 128, T], F32, kind="ExternalOutput").ap()
        dot = nc.dram_tensor("dot", [4, 128, T], F32, kind="ExternalOutput").ap()

    xt = nc.dram_tensor("xt", [D, T], F32, kind="ExternalInput").ap()
    wqt = nc.dram_tensor("wqt", [D, DH], F32, kind="ExternalInput").ap()
    wkt = nc.dram_tensor("wkt", [D, DH], F32, kind="ExternalInput").ap()
    wvt = nc.dram_tensor("wvt", [D, DH], F32, kind="ExternalInput").ap()
    wot = nc.dram_tensor("wot", [DH, D], F32, kind="ExternalInput").ap()
    c2d = nc.dram_tensor("c2", [128, T], F32, kind="ExternalInput").ap()
    s2d = nc.dram_tensor("s2", [128, T], F32, kind="ExternalInput").ap()
    pswapd = nc.dram_tensor("pswap", [128, 128], F32, kind="ExternalInput").ap()
    bdiagd = nc.dram_tensor("bdiag", [128, 128], F32, kind="ExternalInput").ap()
    trid = nc.dram_tensor("trimask", [128, 2048], F32, kind="ExternalInput").ap()
    yt = nc.dram_tensor("yt", [D, T], F32, kind="ExternalOutput").ap()

    with tile.TileContext(nc) as tc, ExitStack() as ctx:
        # ---- persistent pools -------------------------------------------
        qk_pool = ctx.enter_context(tc.tile_pool(name="qk", bufs=1))
        v_pool = ctx.enter_context(tc.tile_pool(name="v", bufs=1))
        const_pool = ctx.enter_context(tc.tile_pool(name="const", bufs=1))

        # Qt/Kt: [128, T] tiles, partition = head-dim (2 heads per tile)
        qt_s = [qk_pool.tile([128, T], F32, name=f"qt{j}") for j in range(4)]
        kt_s = [qk_pool.tile([128, T], F32, name=f"kt{j}") for j in range(4)]
        # V (+ones col): [128, 8*65] per 128-token chunk
        v_s = [v_pool.tile([128, NH * 65], F32, name=f"vt{j}") for j in range(NTS)]
        pswap = const_pool.tile([128, 128], F32, name="pswap_s")
        bdiag = const_pool.tile([128, 128], F32, name="bdiag_s")
        nc.sync.dma_start(_r(pswap[:]), _r(pswapd[:]))
        nc.sync.dma_start(_r(bdiag[:]), _r(bdiagd[:]))
        epsb = const_pool.tile([128, 1], F32, name="epsb")
        nc.gpsimd.memset(epsb[:], 8.0 * EPS)
        onesc = const_pool.tile([128, NH], F32, name="onesc")
        nc.gpsimd.memset(onesc[:], 1.0)
        ones64 = const_pool.tile([128, 1], F32, name="ones64")
        nc.vector.tensor_copy(_r(ones64[:]), onesc[:, 0:1])
        # nk columns: rsqrt(8*(mean+eps)), col = 32*hp + 16*h2 + chunk
        nkcols = const_pool.tile([128, 8 * NTS], F32, name="nkcols")

        # ====== phase 1: QKV projections + rotary + QK-RMSNorm ==========
        with ExitStack() as ph1:
            w_pool = ph1.enter_context(tc.tile_pool(name="wqkv", bufs=1))
            x_pool = ph1.enter_context(tc.tile_pool(name="xs", bufs=10))
            rc_pool = ph1.enter_context(tc.tile_pool(name="rotc", bufs=1))
            t_pool = ph1.enter_context(tc.tile_pool(name="rott", bufs=2))
            ps_p = ph1.enter_context(tc.tile_pool(name="psp", bufs=3, space="PSUM"))
            ps_x = ph1.enter_context(tc.tile_pool(name="psx", bufs=2, space="PSUM"))
            ps_m = ph1.enter_context(tc.tile_pool(name="psm", bufs=2, space="PSUM"))
            ps_nk = ph1.enter_context(
                tc.tile_pool(name="psnk", bufs=1, space="PSUM"))

            wq_s = [w_pool.tile([128, DH], F32, name=f"wq{k}") for k in range(NKC)]
            wk_s = [w_pool.tile([128, DH], F32, name=f"wk{k}") for k in range(NKC)]
            wv_s = [w_pool.tile([128, DH], F32, name=f"wv{k}") for k in range(NKC)]
            for k in range(NKC):
                ksl = slice(k * 128, (k + 1) * 128)
                nc.sync.dma_start(_r(wq_s[k][:]), _r(wqt[ksl, :]))
                nc.sync.dma_start(_r(wk_s[k][:]), _r(wkt[ksl, :]))
                nc.sync.dma_start(_r(wv_s[k][:]), _r(wvt[ksl, :]))
            c2 = rc_pool.tile([128, T], F32, name="c2_s")
            s2 = rc_pool.tile([128, T], F32, name="s2_s")
            nc.sync.dma_start(c2[:], c2d[:, 0:T])
            nc.sync.dma_start(s2[:], s2d[:, 0:T])
            nkp = ps_nk.tile([128, 8 * NTS], F32, name="nkp")

            for tt in range(NTT):
                tsl = slice(tt * 512, (tt + 1) * 512)
                xts = []
                for k in range(NKC):
                    xc = x_pool.tile([128, 512], F32, name="xc", tag="xc")
                    nc.gpsimd.dma_start(_r(xc[:]),
                                        _r(xt[k * 128:(k + 1) * 128, tsl]))
                    xts.append(xc)
                # Qt / Kt: psum[j_loc, t] = sum_d w[d, j] * x[d, t]
                for (wsrc, dst) in ((wq_s, qt_s), (wk_s, kt_s)):
                    for jt in range(4):
                        jsl = slice(jt * 128, (jt + 1) * 128)
                        pp = ps_p.tile([128, 512], F32, name="pp", tag="pp")
                        for k in range(NKC):
                            nc.tensor.matmul(
                                pp[:], _r(wsrc[k][:, jsl]), _r(xts[k][:]),
                                start=(k == 0), stop=(k == NKC - 1))
                        nc.vector.tensor_copy(_r(dst[jt][:, tsl]), pp[:])
                # V: psum[t_loc, j] = sum_d x[d, t] * wv[d, j]
                for ts_ in range(4):
                    ci = tt * 4 + ts_
                    pv = ps_p.tile([128, 512], F32, name="pv", tag="pp")
                    for k in range(NKC):
                        nc.tensor.matmul(
                            pv[:], _r(xts[k][:, ts_ * 128:(ts_ + 1) * 128]),
                            _r(wv_s[k][:]),
                            start=(k == 0), stop=(k == NKC - 1))
                    v3 = v_s[ci].rearrange("p (h e) -> p h e", h=NH)
                    nc.vector.tensor_copy(
                        _r(v3[:, :, 0:64]), pv.rearrange("p (h e) -> p h e", h=NH))
                    nc.vector.tensor_copy(_r(v3[:, :, 64:65]),
                                          onesc[:].unsqueeze(-1))
                # rotary + norm on the just-finished 512-block of each tile
                bsl = tsl
                for jt in range(4):
                    # Q: full norm multiply (nq varies along the scores' free
                    # dim i, so it must be applied to Q itself)
                    q = qt_s[jt]
                    xsq = ps_x.tile([128, 512], F32, name="xsq", tag="xs")
                    nc.tensor.matmul(xsq[:], _r(pswap[:]), _r(q[:, bsl]),
                                     start=True, stop=True)
                    sq = t_pool.tile([128, 512], F32, name="sq", tag="sq")
                    nc.scalar.activation(_r(sq[:]), q[:, bsl], AF.Square)
                    ms = ps_m.tile([128, 512], F32, name="ms", tag="ms")
                    nc.tensor.matmul(ms[:], _r(bdiag[:]), _r(sq[:]),
                                     start=True, stop=True)
                    s1 = t_pool.tile([128, 512], F32, name="s1", tag="s1")
                    nc.scalar.activation(s1[:], ms[:], AF.Sqrt,
                                         scale=0.125, bias=epsb[:])
                    nc.vector.reciprocal_approx_fast(out=s1[:], in_=s1[:])
                    nc.gpsimd.tensor_mul(_r(q[:, bsl]), q[:, bsl], c2[:, bsl])
                    nc.vector.tensor_mul(xsq[:], xsq[:], s2[:, bsl])
                    nc.vector.tensor_add(_r(q[:, bsl]), q[:, bsl], xsq[:])
                    nc.gpsimd.tensor_mul(_r(q[:, bsl]), q[:, bsl], s1[:])
                    # K: rotary only; nk[j] is applied later as exp()'s
                    # per-partition scale. Sumsq via tiny N=1 matmuls.
                    k_ = kt_s[jt]
                    xsk = ps_x.tile([128, 512], F32, name="xsk", tag="xs")
                    nc.tensor.matmul(xsk[:], _r(pswap[:]), _r(k_[:, bsl]),
                                     start=True, stop=True)
                    sqk = t_pool.tile([128, 512], F32, name="sqk", tag="sq")
                    nc.scalar.activation(_r(sqk[:]), k_[:, bsl], AF.Square)
                    for h2 in range(2):
                        for c4 in range(4):
                            col = (2 * jt + h2) * NTS + tt * 4 + c4
                            nc.tensor.matmul(
                                nkp[:, col:col + 1],
                                sqk[h2 * 64:h2 * 64 + 64,
                                    c4 * 128:(c4 + 1) * 128],
                                ones64[h2 * 64:h2 * 64 + 64, :],
                                start=True, stop=True)
                    nc.gpsimd.tensor_mul(_r(k_[:, bsl]), k_[:, bsl], c2[:, bsl])
                    nc.vector.tensor_mul(xsk[:], xsk[:], s2[:, bsl])
                    nc.vector.tensor_add(_r(k_[:, bsl]), k_[:, bsl], xsk[:])
            s1k = t_pool.tile([128, 8 * NTS], F32, name="s1k", tag="s1k")
            nc.scalar.activation(s1k[:], nkp[:], AF.Sqrt,
                                 scale=0.125, bias=epsb[:])
            nc.vector.reciprocal_approx_fast(out=nkcols[:], in_=s1k[:])

        if dbg:  # rotated+normed Q
            for j in range(4):
                nc.sync.dma_start(dqr[j], qt_s[j][:])

        # =================== phase 2: attention =========================
        # Ot assembled, normalized: 4 tiles [128, T] = 512 head-dims
        ot_pool = ctx.enter_context(tc.tile_pool(name="otf", bufs=1))
        wo_pool = ctx.enter_context(tc.tile_pool(name="wo", bufs=1))
        otf = [ot_pool.tile([128, T], F32, name=f"otf{j}") for j in range(4)]
        wot_s = [wo_pool.tile([128, D], F32, name=f"wot{j}") for j in range(4)]
        for k4 in range(4):
            nc.sync.dma_start(_r(wot_s[k4][:]),
                              _r(wot[k4 * 128:(k4 + 1) * 128, :]))
        with ExitStack() as ph2:
            m_pool = ph2.enter_context(tc.tile_pool(name="mask", bufs=1))
            st_pool = ph2.enter_context(tc.tile_pool(name="stg", bufs=3))
            p_pool = ph2.enter_context(tc.tile_pool(name="pexp", bufs=3))
            e_pool = ph2.enter_context(tc.tile_pool(name="epi", bufs=4))
            ps_sc = ph2.enter_context(tc.tile_pool(name="pssc", bufs=1, space="PSUM"))
            ps_ot = ph2.enter_context(tc.tile_pool(name="psot", bufs=4, space="PSUM"))

            tri = m_pool.tile([128, 128], F32, name="tri_s")
            nc.sync.dma_start(tri[:], trid[:, 0:128])
            zer = m_pool.tile([128, 384], F32, name="zer_s")
            nc.gpsimd.memset(zer[:], 0.0)

            # head pairs (2hp, 2hp+1) share a qt/kt tile at partition 0/64;
            # their scores matmuls are emitted adjacently so the PE runs them
            # concurrently in distinct row-groups (K=64 each). i is processed
            # in 1024-wide halves so psum fits deeper pipelining.
            HWID = min(1024, T)
            NBLK = HWID // 512
            for ihalf in range(T // HWID):
                for hp in range(4):
                    ilo = ihalf * HWID
                    nj = (ilo + HWID) // 128   # chunks with j < ihi
                    ot_ps = {(h2, b): ps_ot.tile([128, 512], F32,
                                                 name="otp", tag="otp")
                             for h2 in range(2) for b in range(NBLK)}
                    for jt in range(nj):
                        jsl = slice(jt * 128, (jt + 1) * 128)
                        off0 = max(0, jt * 128 - ilo)  # diag offset in half
                        b0 = off0 // 512               # first live block
                        gap = off0 - b0 * 512
                        scs, ps = {}, {}
                        for h2 in range(2):
                            scs[h2] = ps_sc.tile([128, HWID], F32,
                                                 name="sc", tag=f"sc{h2}")
                        for sub in range(b0, NBLK):
                            ssl = slice(sub * 512, (sub + 1) * 512)
                            isl = slice(ilo + sub * 512, ilo + (sub + 1) * 512)
                            for h2 in range(2):
                                ho = h2 * 64
                                nc.tensor.matmul(
                                    scs[h2][:, ssl],
                                    _r(kt_s[hp][ho:ho + 64, jsl]),
                                    _r(qt_s[hp][ho:ho + 64, isl]),
                                    start=True, stop=True)
                        for h2 in range(2):
                            h = 2 * hp + h2
                            c_ = (2 * hp + h2) * NTS + jt
                            nk = nkcols[:, c_:c_ + 1]
                            p = p_pool.tile([128, HWID], F32, name="p",
                                            tag=f"p{h2}")
                            ps[h2] = p
                            if gap:
                                nc.gpsimd.tensor_copy(
                                    _r(p[:, b0 * 512:b0 * 512 + gap]),
                                    zer[:, 0:gap])
                            nc.scalar.activation(
                                _r(p[:, off0:HWID]), scs[h2][:, off0:HWID],
                                AF.Exp, scale=nk[:])
                            if off0 < HWID and jt * 128 >= ilo:
                                # causal mask on the diagonal 128-wide block
                                nc.gpsimd.tensor_mul(
                                    _r(p[:, off0:off0 + 128]),
                                    p[:, off0:off0 + 128], tri[:, 0:128])
                            if dbg and h == 0 and ihalf == 0 and jt in (0, 5):
                                di = 0 if jt == 0 else 1
                                nc.sync.dma_start(
                                    dp[di][:, b0 * 512:HWID],
                                    p[:, b0 * 512:HWID])
                        for h2 in range(2):
                            h = 2 * hp + h2
                            for b in range(b0, NBLK):
                                ib = NBLK * ihalf + b
                                nc.tensor.matmul(
                                    ot_ps[(h2, b)][0:65, :],
                                    _r(v_s[jt][:, 65 * h:65 * h + 65]),
                                    _r(ps[h2][:, b * 512:(b + 1) * 512]),
                                    start=(jt == 0), stop=(jt == 4 * ib + 3))
                    # epilogue: divide by denominator row (psum row 64)
                    for h2 in range(2):
                        h = 2 * hp + h2
                        ho = h2 * 64
                        for b in range(NBLK):
                            ib = NBLK * ihalf + b
                            op = ot_ps[(h2, b)]
                            # recip_approx_fast mishandles base_partition != 0
                            # on HW: stage the denom row to base-0 sbuf first.
                            den = e_pool.tile([1, 512], F32, name="den",
                                              tag="den")
                            nc.vector.tensor_copy(den[:], op[64:65, :])
                            rden = e_pool.tile([1, 512], F32, name="rden",
                                               tag="rden")
                            nc.vector.reciprocal_approx_fast(out=rden[:],
                                                             in_=den[:])
                            rb = e_pool.tile([64, 512], F32, name="rb",
                                             tag="rb")
                            nc.gpsimd.partition_broadcast(rb[:], rden[:],
                                                          channels=64)
                            nc.vector.tensor_mul(
                                _r(otf[hp][ho:ho + 64,
                                           ib * 512:(ib + 1) * 512]),
                                op[0:64, :], rb[:])

        if dbg:
            for j in range(4):
                nc.sync.dma_start(dot[j], otf[j][:])

        # ================ phase 3: output projection ====================
        with ExitStack() as ph3:
            st3_pool = ph3.enter_context(tc.tile_pool(name="stg3", bufs=3))
            ps_y = ph3.enter_context(tc.tile_pool(name="psy", bufs=3, space="PSUM"))
            for tt in range(NTT):
                tsl = slice(tt * 512, (tt + 1) * 512)
                for dt_ in range(8):
                    dsl = slice(dt_ * 128, (dt_ + 1) * 128)
                    py = ps_y.tile([128, 512], F32, name="py", tag="py")
                    for k4 in range(4):
                        nc.tensor.matmul(py[:], _r(wot_s[k4][:, dsl]),
                                         _r(otf[k4][:, tsl]),
                                         start=(k4 == 0), stop=(k4 == 3))
                    st = st3_pool.tile([128, 512], F32, name="st", tag="st")
                    nc.scalar.copy(st[:], py[:])
                    nc.sync.dma_start(yt[dsl, tsl], st[:])
    return nc


# ---------------- host-side tables & shard prep -------------------------

def host_tables(T: int = 2048):
    n = HD // 4
    af = (1.0 / 1024) ** np.linspace(0, 1, n, dtype=np.float32)
    af = np.concatenate([af, np.zeros(n, np.float32)])  # [32]
    theta = np.outer(np.arange(T, dtype=np.float32), af)  # [T, 32]
    cosT = np.cos(theta).T.astype(np.float32)  # [32, T]
    sinT = np.sin(theta).T.astype(np.float32)
    c2 = np.tile(cosT, (4, 1))                             # [128, T]
    s2 = np.tile(np.concatenate([sinT, -sinT], 0), (2, 1))  # [128, T]
    km = np.arange(128)
    pswap = (km[:, None] == (km[None, :] ^ 32)).astype(np.float32)
    bdiag = ((km[:, None] // 64) == (km[None, :] // 64)).astype(np.float32)
    tri = np.zeros((128, 2048), np.float32)
    r_ = np.arange(128)[:, None]
    c_ = np.arange(512)[None, :]
    for v in range(4):
        tri[:, 512 * v:512 * (v + 1)] = (c_ >= 128 * v + r_).astype(np.float32)
    return {"c2": np.ascontiguousarray(c2), "s2": np.ascontiguousarray(s2),
            "pswap": pswap, "bdiag": bdiag, "trimask": tri}


def core_inputs(x, wq, wk, wv, wo, core: int, T: int = 2048):
    b, g = core % 4, core // 4
    sl = slice(g * DH, (g + 1) * DH)
    m = {
        "xt": np.ascontiguousarray(np.asarray(x[b]).T.astype(np.float32)),
        "wqt": np.ascontiguousarray(np.asarray(wq)[sl, :].T.astype(np.float32)),
        "wkt": np.ascontiguousarray(np.asarray(wk)[sl, :].T.astype(np.float32)),
        "wvt": np.ascontiguousarray(np.asarray(wv)[sl, :].T.astype(np.float32)),
        "wot": np.ascontiguousarray(np.asarray(wo)[:, sl].T.astype(np.float32)),
    }
    m.update(host_tables(T))
    return m


_CACHE = {}


def _get_nc(T: int = 2048):
    key = ("nc", T)
    if key not in _CACHE:
        nc = bacc.Bacc("TRN2", target_bir_lowering=False, debug=False)
        build_kernel(nc, T)
        nc.compile()
        _CACHE[key] = nc
    return _CACHE[key]


def kernel(x, wq, wk, wv, wo, mask=None):
    from concourse import bass_utils
    nc = _get_nc(2048)
    in_maps = [core_inputs(x, wq, wk, wv, wo, c) for c in range(8)]
    res = bass_utils.run_bass_kernel_spmd(nc, in_maps, list(range(8)))
    outs = [np.asarray(res.results[c]["yt"]) for c in range(8)]
    out = np.empty((4, 2048, 1024), np.float32)
    for b in range(4):
        out[b] = (outs[b] + outs[b + 4]).T
    return out



# revision 42
# speedup vs baseline: 96.8419x; 96.8419x over previous
"""Trainium2 Bass kernel: MultiHeadAttention with QK-RMSNorm + partial rotary,
causal softmax. B=4, T=2048, D=1024, H=16, HD=64, fp32.

Sharding: 8 cores = 4 batches x 2 head-groups (8 heads each). Each core:
  - QKV projections for its batch, restricted to its 512 head-dims
  - causal attention for its 8 heads
  - partial output projection (its 512 contraction dims, all 1024 outputs)
Host sums the two head-group partials per batch (the all-reduce) and
transposes back.

Layout: fully transposed pipeline, zero on-chip transposes:
  xt [D, T] -> Qt/Kt [hd, t] (proj with wT as lhsT), V [t, hd]
  scores St[j, i] = Kt^T-row . Qt-col  (transposed scores, causal over j<=i)
  softmax without max-subtraction (RMS-normed q,k bound |s| <= 8)
  AV: lhsT = [V | ones] (j, 65), rhs = exp(St) -> Ot [65, i] with
      row 64 = softmax denominator (free)
  out-proj: lhsT = woT chunk, rhs = normalized Ot -> yt [dout, t]
Matmuls run as float32r (full-rate fp32 on the PE at N>=256).
"""

import numpy as np
from contextlib import ExitStack

import concourse.bass as bass
import concourse.tile as tile
import concourse.mybir as mybir
from concourse import bacc

F32 = mybir.dt.float32
MM_DT = mybir.dt.float32r # float32r = full-rate; float32 = exact, 1/4-rate
AF = mybir.ActivationFunctionType
MULT = mybir.AluOpType.mult
ADD = mybir.AluOpType.add

D = 1024   # model dim
DH = 512   # head-group width per core (8 heads x 64)
NH = 8     # heads per core
HD = 64    # head dim
NKC = D // 128   # k-chunks over model dim
EPS = 1e-6


def _r(ap):
    return ap.bitcast(MM_DT)


def _cast_dve(nc, ap):
    """In-place round f32 -> f32r (DVE) so walrus accepts it as mm input."""
    nc.vector.tensor_copy(ap.bitcast(MM_DT), ap)


def _cast_act(nc, ap):
    nc.scalar.copy(ap.bitcast(MM_DT), ap)


def build_kernel(nc: bass.Bass, T: int = 2048, dbg: bool = False):
    """Trace the per-core program. T parameterized for fast sim smoke tests."""
    NTT = T // 512     # 512-wide t/i blocks
    NTS = T // 128     # 128-wide t/j chunks

    if dbg:
        dqt = nc.dram_tensor("dqt", [4, 128, T], F32, kind="ExternalOutput").ap()
        dkt = nc.dram_tensor("dkt", [4, 128, T], F32, kind="ExternalOutput").ap()
        dqr = nc.dram_tensor("dqr", [4, 128, T], F32, kind="ExternalOutput").ap()
        dv = nc.dram_tensor("dv", [NTS, 128, NH * 65], F32,
                            kind="ExternalOutput").ap()
        dp = nc.dram_tensor("dp", [2, 128, T], F32, kind="ExternalOutput").ap()
        dot = nc.dram_tensor("dot", [4, 128, T], F32, kind="ExternalOutput").ap()

    xt = nc.dram_tensor("xt", [D, T], F32, kind="ExternalInput").ap()
    wqt = nc.dram_tensor("wqt", [D, DH], F32, kind="ExternalInput").ap()
    wkt = nc.dram_tensor("wkt", [D, DH], F32, kind="ExternalInput").ap()
    wvt = nc.dram_tensor("wvt", [D, DH], F32, kind="ExternalInput").ap()
    wot = nc.dram_tensor("wot", [DH, D], F32, kind="ExternalInput").ap()
    c2d = nc.dram_tensor("c2", [128, T], F32, kind="ExternalInput").ap()
    s2d = nc.dram_tensor("s2", [128, T], F32, kind="ExternalInput").ap()
    pswapd = nc.dram_tensor("pswap", [128, 128], F32, kind="ExternalInput").ap()
    bdiagd = nc.dram_tensor("bdiag", [128, 128], F32, kind="ExternalInput").ap()
    trid = nc.dram_tensor("trimask", [128, 2048], F32, kind="ExternalInput").ap()
    yt = nc.dram_tensor("yt", [D, T], F32, kind="ExternalOutput").ap()

    with tile.TileContext(nc) as tc, ExitStack() as ctx:
        # ---- persistent pools -------------------------------------------
        qk_pool = ctx.enter_context(tc.tile_pool(name="qk", bufs=1))
        v_pool = ctx.enter_context(tc.tile_pool(name="v", bufs=1))
        const_pool = ctx.enter_context(tc.tile_pool(name="const", bufs=1))

        # Qt/Kt: [128, T] tiles, partition = head-dim (2 heads per tile)
        qt_s = [qk_pool.tile([128, T], F32, name=f"qt{j}") for j in range(4)]
        kt_s = [qk_pool.tile([128, T], F32, name=f"kt{j}") for j in range(4)]
        # V (+ones col): [128, 8*65] per 128-token chunk
        v_s = [v_pool.tile([128, NH * 65], F32, name=f"vt{j}") for j in range(NTS)]
        pswap = const_pool.tile([128, 128], F32, name="pswap_s")
        bdiag = const_pool.tile([128, 128], F32, name="bdiag_s")
        epsb = const_pool.tile([128, 1], F32, name="epsb")
        nc.gpsimd.memset(epsb[:], 8.0 * EPS)
        onesc = const_pool.tile([128, NH], F32, name="onesc")
        nc.gpsimd.memset(onesc[:], 1.0)
        ones64 = const_pool.tile([128, 1], F32, name="ones64")
        nc.vector.tensor_copy(_r(ones64[:]), onesc[:, 0:1])
        # nk columns: rsqrt(8*(mean+eps)), col = 32*hp + 16*h2 + chunk
        nkcols = const_pool.tile([128, 8 * NTS], F32, name="nkcols")
        tri = const_pool.tile([128, 128], F32, name="tri_s")
        zer = const_pool.tile([128, 384], F32, name="zer_s")

        # ====== phase 1: QKV projections + rotary + QK-RMSNorm ==========
        with ExitStack() as ph1:
            w_pool = ph1.enter_context(tc.tile_pool(name="wqkv", bufs=1))
            x_pool = ph1.enter_context(tc.tile_pool(name="xs", bufs=10))
            rc_pool = ph1.enter_context(tc.tile_pool(name="rotc", bufs=1))
            t_pool = ph1.enter_context(tc.tile_pool(name="rott", bufs=2))
            t1_pool = ph1.enter_context(tc.tile_pool(name="rotsq", bufs=1))
            ps_p = ph1.enter_context(tc.tile_pool(name="psp", bufs=4, space="PSUM"))
            ps_x = ph1.enter_context(tc.tile_pool(name="psx", bufs=2, space="PSUM"))
            ps_m = ph1.enter_context(tc.tile_pool(name="psm", bufs=1, space="PSUM"))
            ps_nk = ph1.enter_context(
                tc.tile_pool(name="psnk", bufs=1, space="PSUM"))

            wq_s = [w_pool.tile([128, DH], F32, name=f"wq{k}") for k in range(NKC)]
            wk_s = [w_pool.tile([128, DH], F32, name=f"wk{k}") for k in range(NKC)]
            wv_s = [w_pool.tile([128, DH], F32, name=f"wv{k}") for k in range(NKC)]
            # weight DMAs split across the SP and ACT hwdge queues so the
            # first-tt matmuls aren't gated on one serial DMA queue
            for k in range(NKC):
                ksl = slice(k * 128, (k + 1) * 128)
                nc.sync.dma_start(_r(wq_s[k][:]), _r(wqt[ksl, :]))
            for k in range(NKC):
                ksl = slice(k * 128, (k + 1) * 128)
                nc.sync.dma_start(_r(wk_s[k][:]), _r(wkt[ksl, :]))
            nc.sync.dma_start(tri[:], trid[:, 0:128])
            c2 = rc_pool.tile([128, T], F32, name="c2_s")
            s2 = rc_pool.tile([128, T], F32, name="s2_s")
            # ACT queue: first x chunks (beats the Pool DGE init), small
            # constants for the tt=0 rotary chain, then wv, then the rest.
            x0_pre = []
            for k in range(2):
                xc = x_pool.tile([128, 512], F32, name="xc", tag="xc")
                nc.scalar.dma_start(_r(xc[:]), _r(xt[k * 128:(k + 1) * 128,
                                                    0:512]))
                x0_pre.append(xc)
            nc.scalar.dma_start(_r(pswap[:]), _r(pswapd[:]))
            nc.scalar.dma_start(_r(bdiag[:]), _r(bdiagd[:]))
            nc.scalar.dma_start(c2[:, 0:512], c2d[:, 0:512])
            nc.scalar.dma_start(s2[:, 0:512], s2d[:, 0:512])
            for k in range(NKC):
                ksl = slice(k * 128, (k + 1) * 128)
                nc.scalar.dma_start(_r(wv_s[k][:]), _r(wvt[ksl, :]))
            if T > 512:
                nc.scalar.dma_start(c2[:, 512:T], c2d[:, 512:T])
                nc.scalar.dma_start(s2[:, 512:T], s2d[:, 512:T])
            nkp = ps_nk.tile([128, 8 * NTS], F32, name="nkp")
            nkp3 = nkp.rearrange("p (h c) -> p h c", h=8)
            nkc3 = nkcols.rearrange("p (h c) -> p h c", h=8)

            def emit_nk(tt):
                # rsqrt of the K sumsq columns for block tt (strided slice
                # across the 8 per-head column groups), so attention on
                # early blocks isn't gated on the whole of phase 1.
                csl = slice(tt * 4, (tt + 1) * 4)
                s1k = t_pool.tile([128, 32], F32, name="s1k", tag="s1k")
                nc.scalar.activation(s1k[:], nkp3[:, :, csl], AF.Sqrt,
                                     scale=0.125, bias=epsb[:])
                nc.vector.reciprocal_approx_fast(out=nkc3[:, :, csl],
                                                 in_=s1k[:])

            def emit_block(tt, xts):
                """QKV projections for 512-token block tt with the
                rotary/QK-norm chains software-pipelined one jt behind the
                matmuls, so every phase-1 PSUM consumer retires within the
                block and attention is never gated on a serial tail."""
                bsl = slice(tt * 512, (tt + 1) * 512)
                sqs = {}

                def q_rot(jt):
                    # Q: full norm multiply (nq varies along the scores'
                    # free dim i, so it must be applied to Q itself)
                    q = qt_s[jt]
                    xsq = ps_x.tile([128, 512], F32, name="xsq", tag="xs")
                    nc.tensor.matmul(xsq[:], _r(pswap[:]), _r(q[:, bsl]),
                                     start=True, stop=True)
                    ms = ps_m.tile([128, 512], F32, name="ms", tag="ms")
                    nc.tensor.matmul(ms[:], _r(bdiag[:]), _r(sqs[jt][0][:]),
                                     start=True, stop=True)
                    s1 = t_pool.tile([128, 512], F32, name="s1", tag="s1")
                    nc.scalar.activation(s1[:], ms[:], AF.Sqrt,
                                         scale=0.125, bias=epsb[:])
                    nc.vector.reciprocal_approx_fast(out=s1[:], in_=s1[:])
                    nc.gpsimd.tensor_mul(_r(q[:, bsl]), q[:, bsl], c2[:, bsl])
                    nc.vector.tensor_mul(xsq[:], xsq[:], s2[:, bsl])
                    nc.vector.tensor_add(_r(q[:, bsl]), q[:, bsl], xsq[:])
                    nc.gpsimd.tensor_mul(_r(q[:, bsl]), q[:, bsl], s1[:])

                def k_rot(jt):
                    # K: rotary only; nk[j] is applied later as exp()'s
                    # per-partition scale. Sumsq via tiny N=1 matmuls.
                    k_ = kt_s[jt]
                    sqk = sqs[jt][1]
                    xsk = ps_x.tile([128, 512], F32, name="xsk", tag="xs")
                    nc.tensor.matmul(xsk[:], _r(pswap[:]), _r(k_[:, bsl]),
                                     start=True, stop=True)
                    for h2 in range(2):
                        for c4 in range(4):
                            col = (2 * jt + h2) * NTS + tt * 4 + c4
                            nc.tensor.matmul(
                                nkp[:, col:col + 1],
                                sqk[h2 * 64:h2 * 64 + 64,
                                    c4 * 128:(c4 + 1) * 128],
                                ones64[h2 * 64:h2 * 64 + 64, :],
                                start=True, stop=True)
                    nc.gpsimd.tensor_mul(_r(k_[:, bsl]), k_[:, bsl],
                                         c2[:, bsl])
                    nc.vector.tensor_mul(xsk[:], xsk[:], s2[:, bsl])
                    nc.vector.tensor_add(_r(k_[:, bsl]), k_[:, bsl], xsk[:])

                # Q section: psum[j_loc, t] = sum_d w[d, j] * x[d, t]
                for jt in range(4):
                    jsl = slice(jt * 128, (jt + 1) * 128)
                    pp = ps_p.tile([128, 512], F32, name="pp", tag="pp")
                    for k in range(NKC):
                        nc.tensor.matmul(
                            pp[:], _r(wq_s[k][:, jsl]), _r(xts[k][:]),
                            start=(k == 0), stop=(k == NKC - 1))
                    nc.vector.tensor_copy(_r(qt_s[jt][:, bsl]), pp[:])
                    sq = t1_pool.tile([128, 512], F32, name="sq",
                                      tag=f"sq{jt}")
                    nc.gpsimd.tensor_mul(_r(sq[:]), qt_s[jt][:, bsl],
                                         qt_s[jt][:, bsl])
                    sqs[jt] = [sq, None]
                    if jt > 0:
                        q_rot(jt - 1)
                kick_prefetch(tt)
                # K section
                for jt in range(4):
                    jsl = slice(jt * 128, (jt + 1) * 128)
                    pp = ps_p.tile([128, 512], F32, name="pp", tag="pp")
                    for k in range(NKC):
                        nc.tensor.matmul(
                            pp[:], _r(wk_s[k][:, jsl]), _r(xts[k][:]),
                            start=(k == 0), stop=(k == NKC - 1))
                    nc.scalar.copy(_r(kt_s[jt][:, bsl]), pp[:])
                    sqk = t1_pool.tile([128, 512], F32, name="sqk",
                                       tag=f"sqk{jt}")
                    nc.gpsimd.tensor_mul(_r(sqk[:]), kt_s[jt][:, bsl],
                                         kt_s[jt][:, bsl])
                    sqs[jt][1] = sqk
                    if jt == 0:
                        q_rot(3)
                    else:
                        k_rot(jt - 1)
                # V section: psum[t_loc, j] = sum_d x[d, t] * wv[d, j]
                for ts_ in range(4):
                    ci = tt * 4 + ts_
                    pv = ps_p.tile([128, 512], F32, name="pv", tag="pp")
                    for k in range(NKC):
                        nc.tensor.matmul(
                            pv[:], _r(xts[k][:, ts_ * 128:(ts_ + 1) * 128]),
                            _r(wv_s[k][:]),
                            start=(k == 0), stop=(k == NKC - 1))
                    v3 = v_s[ci].rearrange("p (h e) -> p h e", h=NH)
                    nc.scalar.copy(
                        _r(v3[:, :, 0:64]),
                        pv.rearrange("p (h e) -> p h e", h=NH))
                    nc.gpsimd.tensor_copy(_r(v3[:, :, 64:65]),
                                          onesc[:].unsqueeze(-1))
                    if ts_ == 0:
                        k_rot(3)
                        emit_nk(tt)

            def fetch_x(tt, engs, skip=0):
                tsl = slice(tt * 512, (tt + 1) * 512)
                xts = []
                for k in range(skip, NKC):
                    xc = x_pool.tile([128, 512], F32, name="xc", tag="xc")
                    engs[k % len(engs)].dma_start(
                        _r(xc[:]), _r(xt[k * 128:(k + 1) * 128, tsl]))
                    xts.append(xc)
                return xts

            # tt=0 x chunks on the Pool swdge queue (SP is busy with wq);
            # later blocks prefetch on SP, one block ahead. The prefetch is
            # kicked from inside emit_block (after the Q section) so the
            # swap DMAs of the current block go out first on SP.
            xts_next = [x0_pre + fetch_x(0, [nc.gpsimd], skip=2)]

            def kick_prefetch(tt):
                if tt + 1 < NTT:
                    xts_next.append(fetch_x(tt + 1, [nc.sync]))

            for tt in range(NTT):
                xts = xts_next.pop(0)
                emit_block(tt, xts)
                if tt == 0:
                    nc.gpsimd.memset(zer[:], 0.0)
            # 1-element exp: pulls the sqrt->exp act-table load into the
            # phase-1 tail instead of the first real softmax exp
            dume = t_pool.tile([1, 1], F32, name="dume", tag="s1k")
            nc.scalar.activation(dume[:], epsb[0:1, 0:1], AF.Exp)

        if dbg:  # rotated+normed Q
            for j in range(4):
                nc.sync.dma_start(dqr[j], qt_s[j][:])

        # =================== phase 2: attention =========================
        # Ot assembled, normalized: 4 tiles [128, T] = 512 head-dims
        ot_pool = ctx.enter_context(tc.tile_pool(name="otf", bufs=1))
        wo_pool = ctx.enter_context(tc.tile_pool(name="wo", bufs=1))
        st3_pool = ctx.enter_context(tc.tile_pool(name="stg3", bufs=3))
        otf = [ot_pool.tile([128, T], F32, name=f"otf{j}") for j in range(4)]
        wot_s = [wo_pool.tile([128, D], F32, name=f"wot{j}") for j in range(4)]
        for k4 in range(4):
            nc.sync.dma_start(_r(wot_s[k4][:]),
                              _r(wot[k4 * 128:(k4 + 1) * 128, :]))
        with ExitStack() as ph2:
            st_pool = ph2.enter_context(tc.tile_pool(name="stg", bufs=3))
            p_pool = ph2.enter_context(tc.tile_pool(name="pexp", bufs=3))
            e_pool = ph2.enter_context(tc.tile_pool(name="epi", bufs=4))
            ps_sc = ph2.enter_context(tc.tile_pool(name="pssc", bufs=1, space="PSUM"))
            ps_ot = ph2.enter_context(tc.tile_pool(name="psot", bufs=4, space="PSUM"))

            # head pairs (2hp, 2hp+1) share a qt/kt tile at partition 0/64;
            # their scores matmuls are emitted adjacently so the PE runs them
            # concurrently in distinct row-groups (K=64 each). i is processed
            # in 1024-wide halves so psum fits deeper pipelining.
            HWID = min(1024, T)
            NBLK = HWID // 512

            def y3_chunk(yps, tt, dt_):
                # one output-projection chunk; py comes from the given psum
                # pool (during ihalf=1 it reuses freed ot_ps slots, turning
                # PE idle at unit boundaries into useful work)
                tsl = slice(tt * 512, (tt + 1) * 512)
                dsl = slice(dt_ * 128, (dt_ + 1) * 128)
                py = yps.tile([128, 512], F32, name="py", tag="otp")
                for k4 in range(4):
                    nc.tensor.matmul(py[:], _r(wot_s[k4][:, dsl]),
                                     _r(otf[k4][:, tsl]),
                                     start=(k4 == 0), stop=(k4 == 3))
                st = st3_pool.tile([128, 512], F32, name="st", tag="st")
                nc.vector.tensor_copy(st[:], py[:])
                nc.sync.dma_start(yt[dsl, tsl], st[:])

            y3_left = [(tt, dt_) for tt in range(NTT) for dt_ in range(8)]
            for ihalf in range(T // HWID):
                for hp in range(4):
                    ilo = ihalf * HWID
                    nj = (ilo + HWID) // 128   # chunks with j < ihi
                    ot_ps = {(h2, b): ps_ot.tile([128, 512], F32,
                                                 name="otp", tag="otp")
                             for h2 in range(2) for b in range(NBLK)}

                    def emit_av(jt_, ps_, b0_):
                        # AV matmuls for jt_, emitted one iteration behind
                        # the scores so exp(jt_) has already run while the
                        # PE was busy with scores(jt_+1).
                        for h2 in range(2):
                            h = 2 * hp + h2
                            for b in range(b0_, NBLK):
                                ib = NBLK * ihalf + b
                                nc.tensor.matmul(
                                    ot_ps[(h2, b)][0:65, :],
                                    _r(v_s[jt_][:, 65 * h:65 * h + 65]),
                                    _r(ps_[h2][:, b * 512:(b + 1) * 512]),
                                    start=(jt_ == 0), stop=(jt_ == 4 * ib + 3))
                        # epilogue per block as soon as its accumulation
                        # closes, overlapping the remaining jt iterations
                        for b in range(b0_, NBLK):
                            ib = NBLK * ihalf + b
                            if jt_ != 4 * ib + 3:
                                continue
                            for h2 in range(2):
                                ho = h2 * 64
                                op = ot_ps[(h2, b)]
                                # recip_approx_fast mishandles
                                # base_partition != 0 on HW: stage the denom
                                # row to base-0 sbuf first.
                                den = e_pool.tile([1, 512], F32, name="den",
                                                  tag="den")
                                nc.vector.tensor_copy(den[:], op[64:65, :])
                                rden = e_pool.tile([1, 512], F32, name="rden",
                                                   tag="rden")
                                nc.vector.reciprocal_approx_fast(out=rden[:],
                                                                 in_=den[:])
                                rb = e_pool.tile([64, 512], F32, name="rb",
                                                 tag="rb")
                                nc.gpsimd.partition_broadcast(rb[:], rden[:],
                                                              channels=64)
                                nc.vector.tensor_mul(
                                    _r(otf[hp][ho:ho + 64,
                                               ib * 512:(ib + 1) * 512]),
                                    op[0:64, :], rb[:])

                    pending = None
                    for jt in range(nj):
                        jsl = slice(jt * 128, (jt + 1) * 128)
                        off0 = max(0, jt * 128 - ilo)  # diag offset in half
                        b0 = off0 // 512               # first live block
                        gap = off0 - b0 * 512
                        scs, ps = {}, {}
                        for h2 in range(2):
                            scs[h2] = ps_sc.tile([128, HWID], F32,
                                                 name="sc", tag=f"sc{h2}")
                        for sub in range(b0, NBLK):
                            ssl = slice(sub * 512, (sub + 1) * 512)
                            isl = slice(ilo + sub * 512, ilo + (sub + 1) * 512)
                            for h2 in range(2):
                                ho = h2 * 64
                                nc.tensor.matmul(
                                    scs[h2][:, ssl],
                                    _r(kt_s[hp][ho:ho + 64, jsl]),
                                    _r(qt_s[hp][ho:ho + 64, isl]),
                                    start=True, stop=True)
                        for h2 in range(2):
                            c_ = (2 * hp + h2) * NTS + jt
                            nk = nkcols[:, c_:c_ + 1]
                            p = p_pool.tile([128, HWID], F32, name="p",
                                            tag=f"p{h2}")
                            ps[h2] = p
                            if gap:
                                nc.gpsimd.tensor_copy(
                                    _r(p[:, b0 * 512:b0 * 512 + gap]),
                                    zer[:, 0:gap])
                            nc.scalar.activation(
                                _r(p[:, off0:HWID]), scs[h2][:, off0:HWID],
                                AF.Exp, scale=nk[:])
                            if off0 < HWID and jt * 128 >= ilo:
                                # causal mask on the diagonal 128-wide block
                                nc.gpsimd.tensor_mul(
                                    _r(p[:, off0:off0 + 128]),
                                    p[:, off0:off0 + 128], tri[:, 0:128])
                        if pending is not None:
                            emit_av(*pending)
                        pending = (jt, ps, b0)
                    emit_av(*pending)
                    if ihalf == T // HWID - 1 and T > 512:
                        # interleave output-projection chunks for the
                        # already-finished i-halves between head-pair units
                        navail = 8 * (NTT - NTT // (T // HWID))
                        take = min(navail // 4, len(y3_left))
                        for _ in range(take):
                            y3_chunk(ps_ot, *y3_left.pop(0))

        if dbg:
            for j in range(4):
                nc.sync.dma_start(dot[j], otf[j][:])

        # ================ phase 3: output projection ====================
        with ExitStack() as ph3:
            ps_y = ph3.enter_context(tc.tile_pool(name="psy", bufs=3, space="PSUM"))
            for (tt, dt_) in y3_left:
                y3_chunk(ps_y, tt, dt_)
    return nc


# ---------------- host-side tables & shard prep -------------------------

def host_tables(T: int = 2048):
    n = HD // 4
    af = (1.0 / 1024) ** np.linspace(0, 1, n, dtype=np.float32)
    af = np.concatenate([af, np.zeros(n, np.float32)])  # [32]
    theta = np.outer(np.arange(T, dtype=np.float32), af)  # [T, 32]
    cosT = np.cos(theta).T.astype(np.float32)  # [32, T]
    sinT = np.sin(theta).T.astype(np.float32)
    c2 = np.tile(cosT, (4, 1))                             # [128, T]
    s2 = np.tile(np.concatenate([sinT, -sinT], 0), (2, 1))  # [128, T]
    km = np.arange(128)
    pswap = (km[:, None] == (km[None, :] ^ 32)).astype(np.float32)
    bdiag = ((km[:, None] // 64) == (km[None, :] // 64)).astype(np.float32)
    tri = np.zeros((128, 2048), np.float32)
    r_ = np.arange(128)[:, None]
    c_ = np.arange(512)[None, :]
    for v in range(4):
        tri[:, 512 * v:512 * (v + 1)] = (c_ >= 128 * v + r_).astype(np.float32)
    return {"c2": np.ascontiguousarray(c2), "s2": np.ascontiguousarray(s2),
            "pswap": pswap, "bdiag": bdiag, "trimask": tri}


def core_inputs(x, wq, wk, wv, wo, core: int, T: int = 2048):
    b, g = core % 4, core // 4
    sl = slice(g * DH, (g + 1) * DH)
    m = {
        "xt": np.ascontiguousarray(np.asarray(x[b]).T.astype(np.float32)),
        "wqt": np.ascontiguousarray(np.asarray(wq)[sl, :].T.astype(np.float32)),
        "wkt": np.ascontiguousarray(np.asarray(wk)[sl, :].T.astype(np.float32)),
        "wvt": np.ascontiguousarray(np.asarray(wv)[sl, :].T.astype(np.float32)),
        "wot": np.ascontiguousarray(np.asarray(wo)[:, sl].T.astype(np.float32)),
    }
    m.update(host_tables(T))
    return m


_CACHE = {}


def _get_nc(T: int = 2048):
    key = ("nc", T)
    if key not in _CACHE:
        nc = bacc.Bacc("TRN2", target_bir_lowering=False, debug=False)
        build_kernel(nc, T)
        nc.compile()
        _CACHE[key] = nc
    return _CACHE[key]


def kernel(x, wq, wk, wv, wo, mask=None):
    from concourse import bass_utils
    nc = _get_nc(2048)
    in_maps = [core_inputs(x, wq, wk, wv, wo, c) for c in range(8)]
    res = bass_utils.run_bass_kernel_spmd(nc, in_maps, list(range(8)))
    outs = [np.asarray(res.results[c]["yt"]) for c in range(8)]
    out = np.empty((4, 2048, 1024), np.float32)
    for b in range(4):
        out[b] = (outs[b] + outs[b + 4]).T
    return out

